# revision 1
# baseline (speedup 1.0000x reference)
"""MLA prefill kernel for Trainium2, 8 NeuronCores.

Sharding: core c -> (batch b = c // 2, head-group g = c % 2). Each core
computes its batch's full sequence for its 8 heads, producing a partial
output (transposed, [2048, 1024]); the host sums the two head-group
partials per batch and transposes back.

Layout strategy (all on-chip matmuls contract over the partition dim):
  x is passed in transposed ([D, L]) per batch.  Down/up projections
  produce latent-major / headdim-major activations directly.  Attention
  runs "k-major": scores^T [k_tok, q_tok] tiles, exp on ACT, denominators
  via ones-matmul column sums, value aggregation (LO^T) needs no P
  transposes.  Softmax max-subtraction is skipped (scores are O(1);
  mathematically identical).  Causality: strictly-upper key blocks are
  skipped (exp underflows to exactly 0 in the reference), diagonal blocks
  masked with affine_select after exp.
"""

import math
import os
from contextlib import ExitStack

import numpy as np

import concourse.bass as bass
import concourse.mybir as mybir
import concourse.tile as tile
from concourse import bacc, bass_utils
from concourse.masks import make_identity

# ---- problem constants -------------------------------------------------
B, L, D = 4, 1024, 2048
H, NOPE, ROPE, VD, KVR = 16, 128, 64, 128, 512
DQ = 1024            # q latent dim
HD = NOPE + ROPE     # 192 per-head q dim
EPS = 1e-6
NH = 8               # heads per core
N_CORES = 8
SCALE = 1.0 / math.sqrt(NOPE + ROPE)

F32 = mybir.dt.float32
F32R = mybir.dt.float32r

USE_F32R = os.environ.get("MLA_F32", "0") != "1"
DT = F32R if USE_F32R else F32
REPS = int(os.environ.get("MLA_REPS", "1"))  # timing amplification only

TOK = 512            # q-token tile (free dim of most matmuls)
NTOK = L // TOK      # 2
KB = 128             # key-token block
NKB = L // KB        # 8
ND = D // 128        # 16 contraction blocks over model dim
NLAT = DQ // 128     # 8 contraction blocks over q latent
NKV = KVR // 128     # 4 blocks over kv latent


def build_nc():
    nc = bacc.Bacc("TRN2", target_bir_lowering=False, debug=False)

    t = {}
    t["x_t"] = nc.dram_tensor("x_t", [D, L], DT, kind="ExternalInput").ap()
    t["wqd_t"] = nc.dram_tensor("wqd_t", [D, DQ], DT, kind="ExternalInput").ap()
    t["wqu_t"] = nc.dram_tensor("wqu_t", [DQ, NH * HD], DT, kind="ExternalInput").ap()
    t["wkvd_t"] = nc.dram_tensor(
        "wkvd_t", [D, KVR + ROPE], DT, kind="ExternalInput"
    ).ap()
    t["wupn"] = nc.dram_tensor("wupn", [NH, NOPE, KVR], DT, kind="ExternalInput").ap()
    t["wupv_t"] = nc.dram_tensor(
        "wupv_t", [NH, 128, NKV, VD], DT, kind="ExternalInput"
    ).ap()
    t["wout_t"] = nc.dram_tensor(
        "wout_t", [NH * VD, D], DT, kind="ExternalInput"
    ).ap()
    t["cosf"] = nc.dram_tensor("cosf", [128, L], F32, kind="ExternalInput").ap()
    t["sinf"] = nc.dram_tensor("sinf", [128, L], F32, kind="ExternalInput").ap()
    t["vn_d"] = nc.dram_tensor("vn_d", [NH, VD, L], DT, kind="Internal").ap()
    t["outT"] = nc.dram_tensor("outT", [D, L], F32, kind="ExternalOutput").ap()

    with tile.TileContext(nc) as tc:
        _emit(tc, t)
    nc.compile()
    return nc


def _emit(tc, t):
    nc = tc.nc
    with ExitStack() as c0:
        c0.enter_context(
            nc.allow_low_precision(reason="fp32r rounding is intentional")
        )
        glob = c0.enter_context(tc.tile_pool(name="glob", bufs=1))
        ps_a = c0.enter_context(tc.tile_pool(name="ps_a", bufs=2, space="PSUM"))
        ps_lo = c0.enter_context(tc.tile_pool(name="ps_lo", bufs=4, space="PSUM"))
        ps_dv = c0.enter_context(tc.tile_pool(name="ps_dv", bufs=2, space="PSUM"))

        # ---- constants ------------------------------------------------
        from concourse import library_config

        nc.gpsimd.load_library(library_config.attnmlp)
        ident = glob.tile([128, 128], F32, tag="ident")
        make_identity(nc, ident)
        ones_f32 = glob.tile([128, 128], F32, tag="ones32")
        nc.vector.memset(ones_f32, 1.0)
        ones_col = glob.tile([128, 1], DT, tag="ones")
        nc.vector.tensor_copy(ones_col, ones_f32[:, :1])
        ones_row = glob.tile([1, 128], DT, tag="onesr")
        nc.vector.tensor_copy(ones_row, ones_f32[:1, :])
        eps_t = glob.tile([1, 1], F32, tag="eps")
        nc.vector.memset(eps_t, EPS)
        cosf = glob.tile([128, L], F32, tag="cosf")
        nc.sync.dma_start(out=cosf, in_=t["cosf"])
        sinf = glob.tile([128, L], F32, tag="sinf")
        nc.sync.dma_start(out=sinf, in_=t["sinf"])

        for _rep in range(REPS):
            with ExitStack() as c1:
                p1 = c1.enter_context(tc.tile_pool(name=f"p1_{_rep}", bufs=1))
                kv_lat = p1.tile([128, NKV, L], DT, tag="kvlat")
                k_roped = p1.tile([128, L], DT, tag="kroped")

                with ExitStack() as c2:
                    p2 = c2.enter_context(
                        tc.tile_pool(name=f"p2_{_rep}", bufs=1)
                    )
                    qT_nope = p2.tile([128, NH, L], DT, tag="qnope")
                    q_roped = p2.tile([128, NH // 2, L], DT, tag="qroped")

                    _emit_front(tc, t, glob, ps_a, ps_dv,
                                ident, ones_col, eps_t, cosf, sinf,
                                kv_lat, k_roped, qT_nope, q_roped, _rep)

                    _emit_attn(tc, t, glob, ps_a, ps_lo, ps_dv,
                               ident, ones_col, ones_row,
                               kv_lat, k_roped, qT_nope, q_roped, _rep)

            _emit_outproj(tc, t, glob, ps_a, _rep)


def _emit_front(tc, t, glob, ps_a, ps_dv, ident, ones_col, eps_t, cosf, sinf,
                kv_lat, k_roped, qT_nope, q_roped, rep=0):
    """Down projections, RMS norms, k-rope, q up-projection + q-rope."""
    nc = tc.nc
    with ExitStack() as c3:
        p3 = c3.enter_context(tc.tile_pool(name=f"p3_{rep}", bufs=1))
        p3s = c3.enter_context(tc.tile_pool(name=f"p3s_{rep}", bufs=3))
        q_lat = p3.tile([128, NLAT, L], DT, tag="qlat")
        kr_pair = p3.tile([128, 2, L], F32, tag="krpair")

        # ---- phase 1: down projections, x streamed in d-halves -------
        with ExitStack() as c4:
            p4 = c4.enter_context(tc.tile_pool(name=f"p4_{rep}", bufs=1))
            p4s = c4.enter_context(tc.tile_pool(name=f"p4s_{rep}", bufs=2))
            wqd_r = t["wqd_t"].rearrange("(b p) m -> p b m", p=128)
            wkvd_r = t["wkvd_t"].rearrange("(b p) m -> p b m", p=128)
            x_r = t["x_t"].rearrange("(b p) t -> p b t", p=128)

            for half in range(2):
                hs = slice(half * 8, half * 8 + 8)
                xh = p4.tile([128, 8, L], DT, tag="xh", bufs=1)
                nc.sync.dma_start(out=xh, in_=x_r[:, hs, :])

                for lb in range(NLAT):
                    wqd = p4s.tile([128, 8, 128], DT, tag="wqd")
                    nc.sync.dma_start(
                        out=wqd, in_=wqd_r[:, hs, lb * 128 : (lb + 1) * 128]
                    )
                    for tk in range(NTOK):
                        ts = slice(tk * TOK, (tk + 1) * TOK)
                        ps = ps_a.tile([128, TOK], F32, tag="a")
                        for db in range(8):
                            nc.tensor.matmul(
                                ps, wqd[:, db, :], xh[:, db, ts],
                                start=(db == 0), stop=(db == 7),
                            )
                        dst = q_lat[:, lb, ts]
                        if half == 0:
                            nc.vector.tensor_copy(dst, ps)
                        else:
                            nc.vector.tensor_add(dst, dst, ps)

                for mb in range(NKV + 1):
                    mw = 128 if mb < NKV else ROPE
                    wkv = p4s.tile([128, 8, 128], DT, tag="wkv")
                    nc.sync.dma_start(
                        out=wkv[:, :, :mw],
                        in_=wkvd_r[:, hs, mb * 128 : mb * 128 + mw],
                    )
                    for tk in range(NTOK):
                        ts = slice(tk * TOK, (tk + 1) * TOK)
                        ps = ps_a.tile([128, TOK], F32, tag="a")
                        for db in range(8):
                            nc.tensor.matmul(
                                ps[:mw], wkv[:, db, :mw], xh[:, db, ts],
                                start=(db == 0), stop=(db == 7),
                            )
                        if mb < NKV:
                            dst = kv_lat[:, mb, ts]
                        else:
                            dst = kr_pair[:ROPE, 0, ts]
                        if half == 0:
                            nc.vector.tensor_copy(dst, ps[:mw])
                        else:
                            nc.vector.tensor_add(dst, dst, ps[:mw])

        # ---- phase 1.5: RMS-normalize q_lat (latent-major) -----------
        rq_row = p3.tile([1, L], F32, tag="rqrow")
        for tk in range(NTOK):
            ts = slice(tk * TOK, (tk + 1) * TOK)
            ps_ssq = ps_dv.tile([1, TOK], F32, tag="dv")
            for lb in range(NLAT):
                sq = p3s.tile([128, TOK], DT, tag="sq")
                sl = q_lat[:, lb, ts]
                nc.vector.tensor_mul(sq, sl, sl)
                nc.tensor.matmul(
                    ps_ssq, ones_col, sq,
                    start=(lb == 0), stop=(lb == NLAT - 1),
                )
            rt = p3s.tile([1, TOK], F32, tag="rt")
            nc.scalar.activation(
                rt, ps_ssq, mybir.ActivationFunctionType.Sqrt,
                bias=eps_t, scale=1.0 / DQ,
            )
            nc.vector.reciprocal(rq_row[:, ts], rt)
        rq_b = p3.tile([128, L], F32, tag="rqb")
        nc.gpsimd.partition_broadcast(rq_b, rq_row)
        for lb in range(NLAT):
            nc.vector.tensor_mul(q_lat[:, lb, :], q_lat[:, lb, :], rq_b)

        # ---- phase 1.6: RMS-normalize kv_lat (latent-major) ----------
        rkv_row = p3.tile([1, L], F32, tag="rkvrow")
        for tk in range(NTOK):
            ts = slice(tk * TOK, (tk + 1) * TOK)
            ps_ssq = ps_dv.tile([1, TOK], F32, tag="dv")
            for lb in range(NKV):
                sq = p3s.tile([128, TOK], DT, tag="sq")
                sl = kv_lat[:, lb, ts]
                nc.vector.tensor_mul(sq, sl, sl)
                nc.tensor.matmul(
                    ps_ssq, ones_col, sq,
                    start=(lb == 0), stop=(lb == NKV - 1),
                )
            rt = p3s.tile([1, TOK], F32, tag="rt")
            nc.scalar.activation(
                rt, ps_ssq, mybir.ActivationFunctionType.Sqrt,
                bias=eps_t, scale=1.0 / KVR,
            )
            nc.vector.reciprocal(rkv_row[:, ts], rt)
        rkv_b = p3.tile([128, L], F32, tag="rkvb")
        nc.gpsimd.partition_broadcast(rkv_b, rkv_row)
        for lb in range(NKV):
            nc.vector.tensor_mul(kv_lat[:, lb, :], kv_lat[:, lb, :], rkv_b)

        # k rope: swap + rope, duplicated into both partition halves
        _rope_pair(nc, kr_pair, cosf, sinf, k_roped, 0)
        nc.sync.dma_start(out=k_roped[ROPE:], in_=k_roped[:ROPE])

        # ---- phase 2: q up-projection + q rope -----------------------
        with ExitStack() as c5:
            p5s = c5.enter_context(tc.tile_pool(name=f"p5s_{rep}", bufs=2))
            wqu_r = t["wqu_t"].rearrange("(b p) m -> p b m", p=128)
            for h in range(NH):
                wqu = p5s.tile([128, NLAT, HD], DT, tag="wqu")
                nc.sync.dma_start(out=wqu, in_=wqu_r[:, :, h * HD : (h + 1) * HD])
                q_pair = p5s.tile([128, 2, L], F32, tag="pair")
                for tk in range(NTOK):
                    ts = slice(tk * TOK, (tk + 1) * TOK)
                    ps_n = ps_a.tile([128, TOK], F32, tag="a")
                    for lb in range(NLAT):
                        nc.tensor.matmul(
                            ps_n, wqu[:, lb, :NOPE], q_lat[:, lb, ts],
                            start=(lb == 0), stop=(lb == NLAT - 1),
                        )
                    nc.vector.tensor_copy(qT_nope[:, h, ts], ps_n)
                    ps_rp = ps_a.tile([128, TOK], F32, tag="a")
                    for lb in range(NLAT):
                        nc.tensor.matmul(
                            ps_rp[:ROPE], wqu[:, lb, NOPE:],
                            q_lat[:, lb, ts],
                            start=(lb == 0), stop=(lb == NLAT - 1),
                        )
                    nc.vector.tensor_copy(q_pair[:ROPE, 0, ts], ps_rp[:ROPE])
                _rope_pair(nc, q_pair, cosf, sinf, q_roped[:, h // 2, :], h % 2)


def _emit_attn(tc, t, glob, ps_a, ps_lo, ps_dv, ident, ones_col, ones_row,
               kv_lat, k_roped, qT_nope, q_roped, rep=0):
    """kv transpose, per-(head, q-tile) attention; v^T written to DRAM."""
    nc = tc.nc
    with ExitStack() as c6:
        p6 = c6.enter_context(tc.tile_pool(name=f"p6_{rep}", bufs=1))
        p6s = c6.enter_context(tc.tile_pool(name=f"p6s_{rep}", bufs=2))
        p6w = c6.enter_context(tc.tile_pool(name=f"p6w_{rep}", bufs=3))

        # transpose normalized kv_lat -> token-major
        kv_tok = p6.tile([128, NKB, KVR], DT, tag="kvtok")
        for kb in range(NKB):
            ps = ps_a.tile([128, KVR], F32, tag="a")
            for lb in range(NKV):
                nc.tensor.transpose(
                    ps[:, lb * 128 : (lb + 1) * 128],
                    kv_lat[:, lb, kb * 128 : (kb + 1) * 128].bitcast(F32),
                    ident,
                )
            nc.vector.tensor_copy(kv_tok[:, kb, :], ps)

        for h in range(NH):
            hb = (h % 2) * 64
            wn = p6s.tile([128, KVR], DT, tag="wupn")
            nc.sync.dma_start(out=wn, in_=t["wupn"][h])
            wv = p6s.tile([128, NKV, VD], DT, tag="wupv")
            nc.sync.dma_start(out=wv, in_=t["wupv_t"][h])

            q_abs = p6.tile([128, NKV, L], DT, tag="qabs", bufs=1)
            for mb in range(NKV):
                for tk in range(NTOK):
                    ts = slice(tk * TOK, (tk + 1) * TOK)
                    ps = ps_a.tile([128, TOK], F32, tag="a")
                    nc.tensor.matmul(
                        ps, wn[:, mb * 128 : (mb + 1) * 128],
                        qT_nope[:, h, ts],
                    )
                    nc.vector.tensor_copy(q_abs[:, mb, ts], ps)

            for tk in range(NTOK):
                ts = slice(tk * TOK, (tk + 1) * TOK)
                nkb = (tk + 1) * (TOK // KB)
                ps_d = ps_dv.tile([1, TOK], F32, tag="dv")
                ps_los = [
                    ps_lo.tile([128, TOK], F32, tag="lo", name=f"pslo{i}")
                    for i in range(NKV)
                ]
                for kb in range(nkb):
                    ks = slice(kb * 128, (kb + 1) * 128)
                    ps_s = ps_a.tile([128, TOK], F32, tag="a")
                    for lb in range(NKV):
                        nc.tensor.matmul(
                            ps_s, kv_lat[:, lb, ks], q_abs[:, lb, ts],
                            start=(lb == 0), stop=False,
                        )
                    nc.tensor.matmul(
                        ps_s, k_roped[hb : hb + ROPE, ks],
                        q_roped[hb : hb + ROPE, h // 2, ts],
                        start=False, stop=True,
                    )
                    e_t = p6w.tile([128, TOK], DT, tag="e")
                    nc.scalar.activation(
                        e_t, ps_s, mybir.ActivationFunctionType.Exp, scale=SCALE
                    )
                    if kb >= tk * (TOK // KB):
                        nc.gpsimd.affine_select(
                            out=e_t, in_=e_t,
                            pattern=[[1, TOK]],
                            compare_op=mybir.AluOpType.is_ge,
                            fill=0.0,
                            base=tk * TOK - kb * 128,
                            channel_multiplier=-1,
                        )
                    nc.tensor.matmul(
                        ps_d, ones_col, e_t,
                        start=(kb == 0), stop=(kb == nkb - 1),
                    )
                    for lb in range(NKV):
                        nc.tensor.matmul(
                            ps_los[lb],
                            kv_tok[:, kb, lb * 128 : (lb + 1) * 128],
                            e_t,
                            start=(kb == 0), stop=(kb == nkb - 1),
                        )
                rd = p6w.tile([1, TOK], DT, tag="rd")
                nc.vector.reciprocal(rd, ps_d)
                ps_b = ps_dv.tile([128, TOK], F32, tag="dv")
                nc.tensor.matmul(ps_b, ones_row, rd)
                rb_sb = p6w.tile([128, TOK], F32, tag="rb")
                nc.vector.tensor_copy(rb_sb, ps_b)
                lo_t = p6.tile([128, NKV, TOK], DT, tag="lot", bufs=1)
                for lb in range(NKV):
                    nc.vector.tensor_copy(lo_t[:, lb, :], ps_los[lb])
                ps_v = ps_dv.tile([128, TOK], F32, tag="dv")
                for lb in range(NKV):
                    nc.tensor.matmul(
                        ps_v, wv[:, lb, :], lo_t[:, lb, :],
                        start=(lb == 0), stop=(lb == NKV - 1),
                    )
                vn = p6w.tile([128, TOK], DT, tag="vn")
                nc.vector.tensor_mul(vn, ps_v, rb_sb)
                nc.sync.dma_start(out=t["vn_d"][h, :, ts], in_=vn)


def _emit_outproj(tc, t, glob, ps_a, rep=0):
    """out^T [D, L] = sum_h Wout_h^T-blocks @ v_norm_h^T, in two d-halves."""
    nc = tc.nc
    with ExitStack() as c7:
        p7 = c7.enter_context(tc.tile_pool(name=f"p7_{rep}", bufs=1))
        p7s = c7.enter_context(tc.tile_pool(name=f"p7s_{rep}", bufs=3))
        wout_r = t["wout_t"].rearrange("(b p) m -> p b m", p=128)

        vn_sb = p7.tile([128, NH, L], DT, tag="vnsb")
        for h in range(NH):
            nc.sync.dma_start(out=vn_sb[:, h, :], in_=t["vn_d"][h])

        for dh in range(2):
            wout = p7.tile([128, NH, D // 2], DT, tag="wout", bufs=2)
            nc.sync.dma_start(
                out=wout, in_=wout_r[:, :, dh * (D // 2) : (dh + 1) * (D // 2)]
            )
            for db in range(D // 256):
                for tk in range(NTOK):
                    ts = slice(tk * TOK, (tk + 1) * TOK)
                    ps = ps_a.tile([128, TOK], F32, tag="a")
                    for h in range(NH):
                        nc.tensor.matmul(
                            ps, wout[:, h, db * 128 : (db + 1) * 128],
                            vn_sb[:, h, ts],
                            start=(h == 0), stop=(h == NH - 1),
                        )
                    o_t = p7s.tile([128, TOK], F32, tag="o")
                    nc.vector.tensor_copy(o_t, ps)
                    row = dh * (D // 2) + db * 128
                    nc.sync.dma_start(out=t["outT"][row : row + 128, ts], in_=o_t)


def _rope_pair(nc, pair, cosf, sinf, out, half):
    """pair[:64,0,:] = v in split re/im layout (re rows 0..31, im rows
    32..63); fill pair[:64,1,:] with the partner rows, then rope into out
    rows [half*64, half*64+64)."""
    hb = half * 64
    nc.sync.dma_start(out=pair[0:32, 1, :], in_=pair[32:ROPE, 0, :])
    nc.sync.dma_start(out=pair[32:ROPE, 1, :], in_=pair[0:32, 0, :])
    if hb:
        nc.sync.dma_start(out=pair[hb : hb + ROPE, :, :], in_=pair[:ROPE, :, :])
    a = pair[hb : hb + ROPE, 0, :]
    b = pair[hb : hb + ROPE, 1, :]
    ob = out[hb : hb + ROPE]
    # out = a*cos + b*sinf'  (sign of the swap folded into sinf)
    nc.vector.tensor_mul(ob, a, cosf[hb : hb + ROPE])
    nc.vector.tensor_mul(pair[hb : hb + ROPE, 0, :], b, sinf[hb : hb + ROPE])
    nc.vector.tensor_add(ob, ob, pair[hb : hb + ROPE, 0, :])


# ======================================================================
# host side
# ======================================================================

_NC_CACHE = {}


def _get_nc():
    key = ("nc", USE_F32R)
    if key not in _NC_CACHE:
        _NC_CACHE[key] = build_nc()
    return _NC_CACHE[key]


def _prep_shared(inputs):
    wq_down = np.asarray(inputs["Wq_down"], np.float32)
    wq_up = np.asarray(inputs["Wq_up"], np.float32)
    wkv_down = np.asarray(inputs["Wkv_down"], np.float32)
    wkv_up = np.asarray(inputs["Wkv_up"], np.float32)
    wout = np.asarray(inputs["Wout"], np.float32)
    rms_q_w = np.asarray(inputs["rms_q_w"], np.float32)
    rms_kv_w = np.asarray(inputs["rms_kv_w"], np.float32)
    freq = np.asarray(inputs["freq_cis"], np.float32)  # [L, 32, 2]

    # split re/im layout for all rope dims: re parts first, then im parts
    rope_perm = np.concatenate(
        [np.arange(0, ROPE, 2), np.arange(1, ROPE, 2)]
    )  # [64]

    wqd_t = np.ascontiguousarray(wq_down.T)  # [D, DQ]
    wkv_down_p = wkv_down.copy()
    wkv_down_p[KVR:] = wkv_down[KVR:][rope_perm]
    wkvd_t = np.ascontiguousarray(wkv_down_p.T)  # [D, 576]

    # rope tables (dim-major, split re/im, duplicated partition halves)
    cos = freq[:, :, 0].T  # [32, L]
    sin = freq[:, :, 1].T
    cosf64 = np.vstack([cos, cos])  # [64, L]
    sinf64 = np.vstack([-sin, sin])
    cosf = np.ascontiguousarray(np.vstack([cosf64, cosf64]))  # [128, L]
    sinf = np.ascontiguousarray(np.vstack([sinf64, sinf64]))

    wq_up3 = (wq_up * rms_q_w[None, :]).reshape(H, HD, DQ)
    wq_up3 = np.concatenate(
        [wq_up3[:, :NOPE, :], wq_up3[:, NOPE:, :][:, rope_perm, :]], axis=1
    )
    wkv_up3 = wkv_up.reshape(H, NOPE + VD, KVR)
    wout3 = wout.reshape(D, H, VD)

    per_g = []
    for g in range(2):
        hs = list(range(g * NH, (g + 1) * NH))
        wqu_t = np.ascontiguousarray(
            wq_up3[hs].reshape(NH * HD, DQ).T
        )  # [DQ, 1536]
        wupn = np.ascontiguousarray(
            wkv_up3[hs, :NOPE, :] * rms_kv_w[None, None, :]
        )  # [8, 128, 512]
        wupv = wkv_up3[hs, NOPE:, :] * rms_kv_w[None, None, :]  # [8, 128, 512]
        # -> lhsT layout per head: [512, 128] -> [4, 128, 128] -> [128, 4, 128]
        wupv_t = np.ascontiguousarray(
            wupv.transpose(0, 2, 1).reshape(NH, NKV, 128, VD).transpose(0, 2, 1, 3)
        )  # [8, 128, 4, 128]
        wout_t = np.ascontiguousarray(
            wout3[:, hs, :].transpose(1, 2, 0).reshape(NH * VD, D)
        )  # [1024, 2048]
        per_g.append(
            {
                "wqd_t": wqd_t,
                "wqu_t": wqu_t,
                "wkvd_t": wkvd_t,
                "wupn": wupn,
                "wupv_t": wupv_t,
                "wout_t": wout_t,
                "cosf": cosf,
                "sinf": sinf,
            }
        )
    return per_g


def make_in_maps(inputs):
    x = np.asarray(inputs["x"], np.float32)
    per_g = _prep_shared(inputs)
    in_maps = []
    for c in range(N_CORES):
        b, g = c // 2, c % 2
        m = dict(per_g[g])
        m["x_t"] = np.ascontiguousarray(x[b].T)
        in_maps.append(m)
    return in_maps


def kernel(**inputs):
    nc = _get_nc()
    in_maps = make_in_maps(inputs)
    res = bass_utils.run_bass_kernel_spmd(
        nc, in_maps, core_ids=list(range(N_CORES))
    ).results
    out = np.empty((B, L, D), np.float32)
    for b in range(B):
        out[b] = (res[2 * b]["outT"] + res[2 * b + 1]["outT"]).T
    return out



# revision 6
# speedup vs baseline: 1.6279x; 1.6279x over previous
"""MLA prefill kernel for Trainium2, 8 NeuronCores (bf16 compute).

Sharding: core c -> (batch b = c // 2, head-group g = c % 2). Each core
computes its batch's full sequence for its 8 heads, producing a partial
output (transposed, [2048, 1024]); the host sums the two head-group
partials per batch and transposes back.

Layout strategy (all on-chip matmuls contract over the partition dim):
  x is passed in transposed ([D, L]) per batch and kept SBUF-resident.
  Down/up projections produce latent-major / headdim-major activations
  directly.  Attention runs "k-major": scores^T [k_tok, q_tok] tiles,
  exp on ACT, denominators via ones-matmul column sums, value
  aggregation (LO^T) needs no P transposes.  Softmax max-subtraction is
  skipped (scores are O(1); mathematically identical).  Causality:
  strictly-upper key blocks are skipped and, within diagonal blocks,
  only the q-columns >= the block's first key are computed (the rest
  would exp-underflow to exactly 0 in the reference); the 128-wide
  diagonal band is masked with a narrow affine_select after exp.

All activations and weights are bf16 (PSUM accumulation stays fp32);
softmax statistics and RMS statistics are computed in fp32.
"""

import math
from contextlib import ExitStack

import ml_dtypes
import numpy as np

import concourse.bass as bass
import concourse.mybir as mybir
import concourse.tile as tile
from concourse import bacc, bass_utils
from concourse.masks import make_identity

# ---- problem constants -------------------------------------------------
B, L, D = 4, 1024, 2048
H, NOPE, ROPE, VD, KVR = 16, 128, 64, 128, 512
DQ = 1024            # q latent dim
HD = NOPE + ROPE     # 192 per-head q dim
EPS = 1e-6
NH = 8               # heads per core
N_CORES = 8
SCALE = 1.0 / math.sqrt(NOPE + ROPE)

F32 = mybir.dt.float32
F32R = mybir.dt.float32r
BF = mybir.dt.bfloat16
BNP = ml_dtypes.bfloat16

TOK = 512            # q-token tile (free dim of most matmuls)
NTOK = L // TOK      # 2
KB = 128             # key-token block
NKB = L // KB        # 8
ND = D // 128        # 16 contraction blocks over model dim
NLAT = DQ // 128     # 8 contraction blocks over q latent
NKV = KVR // 128     # 4 blocks over kv latent


def build_nc():
    nc = bacc.Bacc("TRN2", target_bir_lowering=False, debug=False)

    t = {}
    t["x_t"] = nc.dram_tensor("x_t", [D, L], BF, kind="ExternalInput").ap()
    t["wqd"] = nc.dram_tensor("wqd", [128, NLAT, ND, 128], BF,
                              kind="ExternalInput").ap()
    t["wkvd128"] = nc.dram_tensor("wkvd128", [128, NKV, ND, 128], BF,
                                  kind="ExternalInput").ap()
    t["wkvdr"] = nc.dram_tensor("wkvdr", [128, ND, ROPE], BF,
                                kind="ExternalInput").ap()
    t["wqun"] = nc.dram_tensor("wqun", [128, NH, NLAT, 128], BF,
                               kind="ExternalInput").ap()
    t["wqur"] = nc.dram_tensor("wqur", [128, NH // 2, NLAT, 128], BF,
                               kind="ExternalInput").ap()
    t["wupn"] = nc.dram_tensor("wupn", [128, NH, KVR], BF,
                               kind="ExternalInput").ap()
    t["wupv"] = nc.dram_tensor("wupv", [128, NH, NKV, VD], BF,
                               kind="ExternalInput").ap()
    t["wout"] = nc.dram_tensor("wout", [128, NH, D], BF,
                               kind="ExternalInput").ap()
    t["cosf"] = nc.dram_tensor("cosf", [128, L], BF, kind="ExternalInput").ap()
    t["sinf"] = nc.dram_tensor("sinf", [128, L], BF, kind="ExternalInput").ap()
    t["outT"] = nc.dram_tensor("outT", [D, L], F32, kind="ExternalOutput").ap()

    with tile.TileContext(nc) as tc:
        _emit(tc, t)
    nc.compile()
    return nc


def _emit(tc, t):
    nc = tc.nc
    with ExitStack() as c0:
        c0.enter_context(
            nc.allow_low_precision(reason="bf16 rounding is intentional")
        )
        glob = c0.enter_context(tc.tile_pool(name="glob", bufs=1))
        ps_a = c0.enter_context(tc.tile_pool(name="ps_a", bufs=2, space="PSUM"))
        ps_lo = c0.enter_context(tc.tile_pool(name="ps_lo", bufs=4, space="PSUM"))
        ps_dv = c0.enter_context(tc.tile_pool(name="ps_dv", bufs=2, space="PSUM"))

        # ---- constants ------------------------------------------------
        from concourse import library_config

        nc.gpsimd.load_library(library_config.attnmlp)
        ident = glob.tile([128, 128], F32, tag="ident")
        make_identity(nc, ident)
        identb = glob.tile([128, 128], BF, tag="identb")
        nc.vector.tensor_copy(identb, ident)
        ones_f32 = glob.tile([128, 128], F32, tag="ones32")
        nc.vector.memset(ones_f32, 1.0)
        ones_col = glob.tile([128, 1], BF, tag="ones")
        nc.vector.tensor_copy(ones_col, ones_f32[:, :1])
        ones_row_r = glob.tile([1, 128], F32R, tag="onesr")
        nc.vector.tensor_copy(ones_row_r, ones_f32[:1, :])
        eps_t = glob.tile([1, 1], F32, tag="eps")
        nc.vector.memset(eps_t, EPS)
        cosf = glob.tile([128, L], BF, tag="cosf")
        nc.sync.dma_start(out=cosf, in_=t["cosf"])
        sinf = glob.tile([128, L], BF, tag="sinf")
        nc.sync.dma_start(out=sinf, in_=t["sinf"])

        with ExitStack() as c1:
            p1 = c1.enter_context(tc.tile_pool(name="p1", bufs=1))
            kv_lat = p1.tile([128, NKV, L], BF, tag="kvlat")
            k_roped = p1.tile([128, L], BF, tag="kroped")
            # wout prefetch begins early; consumed only in the final phase
            wout_sb = p1.tile([128, NH, D], BF, tag="wout")
            vn_sb = p1.tile([128, NH, L], BF, tag="vnsb")

            with ExitStack() as c2:
                p2 = c2.enter_context(tc.tile_pool(name="p2", bufs=1))
                qT_nope = p2.tile([128, NH, L], BF, tag="qnope")
                q_roped = p2.tile([128, NH // 2, L], BF, tag="qroped")

                _emit_front(tc, t, glob, ps_a, ps_dv,
                            ident, ones_col, ones_row_r, eps_t, cosf, sinf,
                            kv_lat, k_roped, qT_nope, q_roped)

                nc.sync.dma_start(out=wout_sb, in_=t["wout"])

                _emit_attn(tc, t, glob, ps_a, ps_lo, ps_dv,
                           identb, ones_col, ones_row_r,
                           kv_lat, k_roped, qT_nope, q_roped, vn_sb)

        _emit_outproj(tc, t, glob, ps_a, wout_sb, vn_sb)


def _rms_block(tc, glob, ps_dv, p3s, lat, nblk, inv_dim, ones_col,
               ones_row_r, eps_t, tag):
    """RMS-normalize latent-major `lat` [128, nblk, L] in place."""
    nc = tc.nc
    r_row = p3s.tile([1, L], F32, tag=f"rrow{tag}", bufs=1)
    for tk in range(NTOK):
        ts = slice(tk * TOK, (tk + 1) * TOK)
        ps_ssq = ps_dv.tile([1, TOK], F32, tag="dv")
        for lb in range(nblk):
            sq = p3s.tile([128, TOK], BF, tag="sq")
            sl = lat[:, lb, ts]
            nc.vector.tensor_mul(sq, sl, sl)
            nc.tensor.matmul(
                ps_ssq, ones_col, sq,
                start=(lb == 0), stop=(lb == nblk - 1),
            )
        rt = p3s.tile([1, TOK], F32, tag="rt")
        nc.scalar.activation(
            rt, ps_ssq, mybir.ActivationFunctionType.Sqrt,
            bias=eps_t, scale=inv_dim,
        )
        nc.vector.reciprocal_approx_fast(out=r_row[:, ts], in_=rt)
    rbf = p3s.tile([128, L], F32, tag=f"rbf{tag}", bufs=1)
    nc.gpsimd.partition_broadcast(rbf, r_row)
    r_b = p3s.tile([128, L], BF, tag=f"rb{tag}", bufs=1)
    nc.vector.tensor_copy(r_b, rbf)
    for lb in range(nblk):
        nc.vector.tensor_mul(lat[:, lb, :], lat[:, lb, :], r_b)


def _emit_front(tc, t, glob, ps_a, ps_dv, ident, ones_col, ones_row_r,
                eps_t, cosf, sinf, kv_lat, k_roped, qT_nope, q_roped):
    """Down projections, RMS norms, k-rope, q up-projection + q-rope."""
    nc = tc.nc
    with ExitStack() as c3:
        p3 = c3.enter_context(tc.tile_pool(name="p3", bufs=1))
        p3s = c3.enter_context(tc.tile_pool(name="p3s", bufs=3))
        q_lat = p3.tile([128, NLAT, L], BF, tag="qlat")
        kr_pair = p3.tile([128, 2, L], BF, tag="krpair")

        # ---- phase 1: down projections, x SBUF-resident --------------
        with ExitStack() as c4:
            p4 = c4.enter_context(tc.tile_pool(name="p4", bufs=1))
            p4s = c4.enter_context(tc.tile_pool(name="p4s", bufs=3))
            x_r = t["x_t"].rearrange("(b p) t -> p b t", p=128)
            x_sb = p4.tile([128, ND, L], BF, tag="xsb")
            for ch in range(4):
                cs = slice(ch * 4, ch * 4 + 4)
                nc.sync.dma_start(out=x_sb[:, cs, :], in_=x_r[:, cs, :])

            for lb in range(NLAT):
                wqd = p4s.tile([128, ND, 128], BF, tag="wqd")
                nc.sync.dma_start(out=wqd, in_=t["wqd"][:, lb])
                for tk in range(NTOK):
                    ts = slice(tk * TOK, (tk + 1) * TOK)
                    ps = ps_a.tile([128, TOK], F32, tag="a")
                    for db in range(ND):
                        nc.tensor.matmul(
                            ps, wqd[:, db, :], x_sb[:, db, ts],
                            start=(db == 0), stop=(db == ND - 1),
                        )
                    nc.vector.tensor_copy(q_lat[:, lb, ts], ps)

            for mb in range(NKV):
                wkv = p4s.tile([128, ND, 128], BF, tag="wkv")
                nc.sync.dma_start(out=wkv, in_=t["wkvd128"][:, mb])
                for tk in range(NTOK):
                    ts = slice(tk * TOK, (tk + 1) * TOK)
                    ps = ps_a.tile([128, TOK], F32, tag="a")
                    for db in range(ND):
                        nc.tensor.matmul(
                            ps, wkv[:, db, :], x_sb[:, db, ts],
                            start=(db == 0), stop=(db == ND - 1),
                        )
                    nc.vector.tensor_copy(kv_lat[:, mb, ts], ps)

            wkr = p4s.tile([128, ND, ROPE], BF, tag="wkr")
            nc.sync.dma_start(out=wkr, in_=t["wkvdr"])
            for tk in range(NTOK):
                ts = slice(tk * TOK, (tk + 1) * TOK)
                ps = ps_a.tile([128, TOK], F32, tag="a")
                for db in range(ND):
                    nc.tensor.matmul(
                        ps[:ROPE], wkr[:, db, :], x_sb[:, db, ts],
                        start=(db == 0), stop=(db == ND - 1),
                    )
                nc.vector.tensor_copy(kr_pair[:ROPE, 0, ts], ps[:ROPE])

        # ---- phase 1.5: RMS-normalize q_lat and kv_lat ---------------
        _rms_block(tc, glob, ps_dv, p3s, q_lat, NLAT, 1.0 / DQ,
                   ones_col, ones_row_r, eps_t, "q")
        _rms_block(tc, glob, ps_dv, p3s, kv_lat, NKV, 1.0 / KVR,
                   ones_col, ones_row_r, eps_t, "kv")

        # k rope: swap + rope (rows 0..63), duplicated into both halves
        _rope_half(nc, kr_pair, cosf, sinf, k_roped)
        nc.sync.dma_start(out=k_roped[ROPE:], in_=k_roped[:ROPE])

        # ---- phase 2: q up-projection + q rope (rope packed 2 heads) -
        with ExitStack() as c5:
            p5s = c5.enter_context(tc.tile_pool(name="p5s", bufs=2))
            for hp in range(NH // 2):
                for hh in range(2):
                    h = 2 * hp + hh
                    wn = p5s.tile([128, NLAT, 128], BF, tag="wqun")
                    nc.sync.dma_start(out=wn, in_=t["wqun"][:, h])
                    for tk in range(NTOK):
                        ts = slice(tk * TOK, (tk + 1) * TOK)
                        ps_n = ps_a.tile([128, TOK], F32, tag="a")
                        for lb in range(NLAT):
                            nc.tensor.matmul(
                                ps_n, wn[:, lb, :], q_lat[:, lb, ts],
                                start=(lb == 0), stop=(lb == NLAT - 1),
                            )
                        nc.vector.tensor_copy(qT_nope[:, h, ts], ps_n)
                wr = p5s.tile([128, NLAT, 128], BF, tag="wqur")
                nc.sync.dma_start(out=wr, in_=t["wqur"][:, hp])
                q_pair = p5s.tile([128, 2, L], BF, tag="pair")
                for tk in range(NTOK):
                    ts = slice(tk * TOK, (tk + 1) * TOK)
                    ps_rp = ps_a.tile([128, TOK], F32, tag="a")
                    for lb in range(NLAT):
                        nc.tensor.matmul(
                            ps_rp, wr[:, lb, :], q_lat[:, lb, ts],
                            start=(lb == 0), stop=(lb == NLAT - 1),
                        )
                    nc.vector.tensor_copy(q_pair[:, 0, ts], ps_rp)
                _rope_full(nc, q_pair, cosf, sinf, q_roped[:, hp, :])


def _emit_attn(tc, t, glob, ps_a, ps_lo, ps_dv, identb, ones_col,
               ones_row_r, kv_lat, k_roped, qT_nope, q_roped, vn_sb):
    """kv transpose, per-(head, q-tile) attention; v^T kept in SBUF."""
    nc = tc.nc
    with ExitStack() as c6:
        p6 = c6.enter_context(tc.tile_pool(name="p6", bufs=1))
        p6a = c6.enter_context(tc.tile_pool(name="p6a", bufs=2))
        p6s = c6.enter_context(tc.tile_pool(name="p6s", bufs=2))
        p6w = c6.enter_context(tc.tile_pool(name="p6w", bufs=3))

        # transpose normalized kv_lat -> token-major
        kv_tok = p6.tile([128, NKB, KVR], BF, tag="kvtok")
        for kb in range(NKB):
            ps = ps_a.tile([128, KVR], BF, tag="a")
            for lb in range(NKV):
                nc.tensor.transpose(
                    ps[:, lb * 128 : (lb + 1) * 128],
                    kv_lat[:, lb, kb * 128 : (kb + 1) * 128],
                    identb,
                )
            nc.vector.tensor_copy(kv_tok[:, kb, :], ps)

        for h in range(NH):
            hb = (h % 2) * 64
            wn = p6s.tile([128, KVR], BF, tag="wupn")
            nc.sync.dma_start(out=wn, in_=t["wupn"][:, h])
            wv = p6s.tile([128, NKV, VD], BF, tag="wupv")
            nc.sync.dma_start(out=wv, in_=t["wupv"][:, h])

            q_abs = p6a.tile([128, NKV, L], BF, tag="qabs")
            for mb in range(NKV):
                for tk in range(NTOK):
                    ts = slice(tk * TOK, (tk + 1) * TOK)
                    ps = ps_a.tile([128, TOK], F32, tag="a")
                    nc.tensor.matmul(
                        ps, wn[:, mb * 128 : (mb + 1) * 128],
                        qT_nope[:, h, ts],
                    )
                    nc.vector.tensor_copy(q_abs[:, mb, ts], ps)

            for tk in range(NTOK):
                ts = slice(tk * TOK, (tk + 1) * TOK)
                nkb = (tk + 1) * (TOK // KB)
                ps_d = ps_dv.tile([1, TOK], F32, tag="dv")
                ps_los = [
                    ps_lo.tile([128, TOK], F32, tag="lo", name=f"pslo{i}")
                    for i in range(NKV)
                ]
                for kb in range(nkb):
                    ks = slice(kb * 128, (kb + 1) * 128)
                    off = max(0, kb * KB - tk * TOK)
                    qs = slice(tk * TOK + off, (tk + 1) * TOK)
                    ps_s = ps_a.tile([128, TOK], F32, tag="a")
                    for lb in range(NKV):
                        nc.tensor.matmul(
                            ps_s[:, off:], kv_lat[:, lb, ks],
                            q_abs[:, lb, qs],
                            start=(lb == 0), stop=False,
                        )
                    nc.tensor.matmul(
                        ps_s[:, off:], k_roped[hb : hb + ROPE, ks],
                        q_roped[hb : hb + ROPE, h // 2, qs],
                        start=False, stop=True,
                    )
                    e_t = p6w.tile([128, TOK], BF, tag="e")
                    nc.scalar.activation(
                        e_t[:, off:], ps_s[:, off:],
                        mybir.ActivationFunctionType.Exp, scale=SCALE,
                    )
                    if kb * KB >= tk * TOK:
                        nc.gpsimd.affine_select(
                            out=e_t[:, off : off + KB],
                            in_=e_t[:, off : off + KB],
                            pattern=[[1, KB]],
                            compare_op=mybir.AluOpType.is_ge,
                            fill=0.0,
                            base=0,
                            channel_multiplier=-1,
                        )
                    nc.tensor.matmul(
                        ps_d[:, off:], ones_col, e_t[:, off:],
                        start=(kb == 0), stop=(kb == nkb - 1),
                    )
                    for lb in range(NKV):
                        nc.tensor.matmul(
                            ps_los[lb][:, off:],
                            kv_tok[:, kb, lb * 128 : (lb + 1) * 128],
                            e_t[:, off:],
                            start=(kb == 0), stop=(kb == nkb - 1),
                        )
                rd = p6w.tile([1, TOK], F32, tag="rd")
                nc.vector.reciprocal_approx_fast(out=rd, in_=ps_d)
                rb_f = p6w.tile([128, TOK], F32, tag="rbf")
                nc.gpsimd.partition_broadcast(rb_f, rd)
                lo_t = p6w.tile([128, NKV, TOK], BF, tag="lot", bufs=2)
                for lb in range(NKV):
                    nc.scalar.copy(lo_t[:, lb, :], ps_los[lb])
                ps_v = ps_dv.tile([128, TOK], F32, tag="dv")
                for lb in range(NKV):
                    nc.tensor.matmul(
                        ps_v, wv[:, lb, :], lo_t[:, lb, :],
                        start=(lb == 0), stop=(lb == NKV - 1),
                    )
                nc.vector.tensor_mul(vn_sb[:, h, ts], ps_v, rb_f)


def _emit_outproj(tc, t, glob, ps_a, wout_sb, vn_sb):
    """out^T [D, L] = sum_h Wout_h^T-blocks @ v_norm_h^T."""
    nc = tc.nc
    with ExitStack() as c7:
        p7s = c7.enter_context(tc.tile_pool(name="p7s", bufs=3))
        for db in range(D // 128):
            for tk in range(NTOK):
                ts = slice(tk * TOK, (tk + 1) * TOK)
                ps = ps_a.tile([128, TOK], F32, tag="a")
                for h in range(NH):
                    nc.tensor.matmul(
                        ps, wout_sb[:, h, db * 128 : (db + 1) * 128],
                        vn_sb[:, h, ts],
                        start=(h == 0), stop=(h == NH - 1),
                    )
                o_t = p7s.tile([128, TOK], F32, tag="o")
                nc.vector.tensor_copy(o_t, ps)
                nc.sync.dma_start(
                    out=t["outT"][db * 128 : (db + 1) * 128, ts], in_=o_t
                )


def _rope_half(nc, pair, cosf, sinf, out):
    """pair[:64,0,:] = v in split re/im layout (re rows 0..31, im rows
    32..63); fill pair[:64,1,:] with the partner rows, then rope into
    out rows [0, 64)."""
    nc.sync.dma_start(out=pair[0:32, 1, :], in_=pair[32:64, 0, :])
    nc.sync.dma_start(out=pair[32:64, 1, :], in_=pair[0:32, 0, :])
    a = pair[:ROPE, 0, :]
    b = pair[:ROPE, 1, :]
    ob = out[:ROPE]
    # out = a*cos + b*sinf'  (sign of the swap folded into sinf)
    nc.vector.tensor_mul(ob, a, cosf[:ROPE])
    nc.vector.tensor_mul(pair[:ROPE, 0, :], b, sinf[:ROPE])
    nc.vector.tensor_add(ob, ob, pair[:ROPE, 0, :])


def _rope_full(nc, pair, cosf, sinf, out):
    """Rope both 64-row halves of pair[:,0,:] (two heads packed) into
    out [128, L]."""
    nc.sync.dma_start(out=pair[0:32, 1, :], in_=pair[32:64, 0, :])
    nc.sync.dma_start(out=pair[32:64, 1, :], in_=pair[0:32, 0, :])
    nc.sync.dma_start(out=pair[64:96, 1, :], in_=pair[96:128, 0, :])
    nc.sync.dma_start(out=pair[96:128, 1, :], in_=pair[64:96, 0, :])
    a = pair[:, 0, :]
    b = pair[:, 1, :]
    nc.vector.tensor_mul(out, a, cosf)
    nc.vector.tensor_mul(pair[:, 0, :], b, sinf)
    nc.vector.tensor_add(out, out, pair[:, 0, :])


# ======================================================================
# host side
# ======================================================================

_NC_CACHE = {}


def _get_nc():
    if "nc" not in _NC_CACHE:
        _NC_CACHE["nc"] = build_nc()
    return _NC_CACHE["nc"]


def _prep_shared(inputs):
    wq_down = np.asarray(inputs["Wq_down"], np.float32)
    wq_up = np.asarray(inputs["Wq_up"], np.float32)
    wkv_down = np.asarray(inputs["Wkv_down"], np.float32)
    wkv_up = np.asarray(inputs["Wkv_up"], np.float32)
    wout = np.asarray(inputs["Wout"], np.float32)
    rms_q_w = np.asarray(inputs["rms_q_w"], np.float32)
    rms_kv_w = np.asarray(inputs["rms_kv_w"], np.float32)
    freq = np.asarray(inputs["freq_cis"], np.float32)  # [L, 32, 2]

    def b16(a):
        return np.ascontiguousarray(a.astype(BNP))

    # split re/im layout for all rope dims: re parts first, then im parts
    rope_perm = np.concatenate(
        [np.arange(0, ROPE, 2), np.arange(1, ROPE, 2)]
    )  # [64]

    # wqd [128, NLAT, ND, 128]: arr[p, lb, b, c] = wq_down.T[b*128+p, lb*128+c]
    wqd_t = wq_down.T  # [D, DQ]
    wqd_h = wqd_t.reshape(ND, 128, NLAT, 128).transpose(1, 2, 0, 3)

    wkv_down_p = wkv_down.copy()
    wkv_down_p[KVR:] = wkv_down[KVR:][rope_perm]
    wkvd_t = wkv_down_p.T  # [D, 576]
    a = wkvd_t.reshape(ND, 128, KVR + ROPE)
    wkvd128_h = a[:, :, :KVR].reshape(ND, 128, NKV, 128).transpose(1, 2, 0, 3)
    wkvdr_h = a[:, :, KVR:].transpose(1, 0, 2)  # [128, ND, 64]

    # rope tables (dim-major, split re/im, duplicated partition halves)
    cos = freq[:, :, 0].T  # [32, L]
    sin = freq[:, :, 1].T
    cosf64 = np.vstack([cos, cos])  # [64, L]
    sinf64 = np.vstack([-sin, sin])
    cosf = np.vstack([cosf64, cosf64])  # [128, L]
    sinf = np.vstack([sinf64, sinf64])

    wq_up3 = (wq_up * rms_q_w[None, :]).reshape(H, HD, DQ)
    wq_up3 = np.concatenate(
        [wq_up3[:, :NOPE, :], wq_up3[:, NOPE:, :][:, rope_perm, :]], axis=1
    )
    wkv_up3 = wkv_up.reshape(H, NOPE + VD, KVR)
    wout3 = wout.reshape(D, H, VD)

    per_g = []
    for g in range(2):
        hs = list(range(g * NH, (g + 1) * NH))
        wn3 = wq_up3[hs, :NOPE, :]  # [8, 128, DQ]
        # wqun [128, NH, NLAT, 128]: arr[p, h, lb, c] = wn3[h, c, lb*128+p]
        wqun_h = (
            wn3.transpose(2, 0, 1)
            .reshape(NLAT, 128, NH, NOPE)
            .transpose(1, 2, 0, 3)
        )
        rg = wq_up3[hs, NOPE:, :]  # [8, 64, DQ]
        bpair = np.stack(
            [np.concatenate([rg[2 * i], rg[2 * i + 1]], axis=0)
             for i in range(NH // 2)]
        )  # [4, 128, DQ]
        wqur_h = (
            bpair.transpose(2, 0, 1)
            .reshape(NLAT, 128, NH // 2, 128)
            .transpose(1, 2, 0, 3)
        )
        wupn = wkv_up3[hs, :NOPE, :] * rms_kv_w[None, None, :]  # [8,128,512]
        wupn_h = wupn.transpose(1, 0, 2)  # [128, NH, 512]
        wupv = wkv_up3[hs, NOPE:, :] * rms_kv_w[None, None, :]  # [8,128,512]
        # wupv [128, NH, NKV, VD]: arr[p, h, lb, v] = wupv[h, v, lb*128+p]
        wupv_h = (
            wupv.transpose(2, 0, 1)
            .reshape(NKV, 128, NH, VD)
            .transpose(1, 2, 0, 3)
        )
        wout_t = wout3[:, hs, :].transpose(1, 2, 0).reshape(NH * VD, D)
        wout_h = wout_t.reshape(NH, 128, D).transpose(1, 0, 2)  # [128,NH,D]
        per_g.append(
            {
                "wqd": b16(wqd_h),
                "wkvd128": b16(wkvd128_h),
                "wkvdr": b16(wkvdr_h),
                "wqun": b16(wqun_h),
                "wqur": b16(wqur_h),
                "wupn": b16(wupn_h),
                "wupv": b16(wupv_h),
                "wout": b16(wout_h),
                "cosf": b16(cosf),
                "sinf": b16(sinf),
            }
        )
    return per_g


def make_in_maps(inputs):
    x = np.asarray(inputs["x"], np.float32)
    per_g = _prep_shared(inputs)
    in_maps = []
    for c in range(N_CORES):
        b, g = c // 2, c % 2
        m = dict(per_g[g])
        m["x_t"] = np.ascontiguousarray(x[b].T.astype(BNP))
        in_maps.append(m)
    return in_maps


def kernel(**inputs):
    nc = _get_nc()
    in_maps = make_in_maps(inputs)
    res = bass_utils.run_bass_kernel_spmd(
        nc, in_maps, core_ids=list(range(N_CORES))
    ).results
    out = np.empty((B, L, D), np.float32)
    for b in range(B):
        out[b] = (res[2 * b]["outT"] + res[2 * b + 1]["outT"]).T
    return out


# revision 12
# speedup vs baseline: 1.6542x; 1.0161x over previous
"""MLA prefill kernel for Trainium2, 8 NeuronCores (bf16 compute).

Sharding: core c -> (batch b = c // 2, head-group g = c % 2). Each core
computes its batch's full sequence for its 8 heads, producing a partial
output (transposed, [2048, 1024]); the host sums the two head-group
partials per batch and transposes back.

Layout strategy (all on-chip matmuls contract over the partition dim):
  x is passed in transposed ([D, L]) per batch and kept SBUF-resident.
  Down/up projections produce latent-major / headdim-major activations
  directly.  Attention runs "k-major": scores^T [k_tok, q_tok] tiles,
  exp on ACT, denominators via ones-matmul column sums, value
  aggregation (LO^T) needs no P transposes.  Softmax max-subtraction is
  skipped (scores are O(1); mathematically identical).  Causality:
  strictly-upper key blocks are skipped and, within diagonal blocks,
  only the q-columns >= the block's first key are computed (the rest
  would exp-underflow to exactly 0 in the reference); the 128-wide
  diagonal band is masked with a narrow affine_select after exp.

All activations and weights are bf16 (PSUM accumulation stays fp32);
softmax statistics and RMS statistics are computed in fp32.
"""

import math
from contextlib import ExitStack

import ml_dtypes
import numpy as np

import concourse.bass as bass
import concourse.mybir as mybir
import concourse.tile as tile
from concourse import bacc, bass_utils
from concourse.masks import make_identity

# ---- problem constants -------------------------------------------------
B, L, D = 4, 1024, 2048
H, NOPE, ROPE, VD, KVR = 16, 128, 64, 128, 512
DQ = 1024            # q latent dim
HD = NOPE + ROPE     # 192 per-head q dim
EPS = 1e-6
NH = 8               # heads per core
N_CORES = 8
SCALE = 1.0 / math.sqrt(NOPE + ROPE)

F32 = mybir.dt.float32
F32R = mybir.dt.float32r
BF = mybir.dt.bfloat16
BNP = ml_dtypes.bfloat16

TOK = 512            # q-token tile (free dim of most matmuls)
NTOK = L // TOK      # 2
KB = 128             # key-token block
NKB = L // KB        # 8
ND = D // 128        # 16 contraction blocks over model dim
NLAT = DQ // 128     # 8 contraction blocks over q latent
NKV = KVR // 128     # 4 blocks over kv latent


def build_nc():
    nc = bacc.Bacc("TRN2", target_bir_lowering=False, debug=False)

    t = {}
    t["x_t"] = nc.dram_tensor("x_t", [D, L], BF, kind="ExternalInput").ap()
    t["wqd"] = nc.dram_tensor("wqd", [128, NLAT, ND, 128], BF,
                              kind="ExternalInput").ap()
    t["wkvd128"] = nc.dram_tensor("wkvd128", [128, NKV, ND, 128], BF,
                                  kind="ExternalInput").ap()
    t["wkvdr"] = nc.dram_tensor("wkvdr", [128, ND, ROPE], BF,
                                kind="ExternalInput").ap()
    t["wqun"] = nc.dram_tensor("wqun", [128, NH, NLAT, 128], BF,
                               kind="ExternalInput").ap()
    t["wqur"] = nc.dram_tensor("wqur", [128, NH // 2, NLAT, 128], BF,
                               kind="ExternalInput").ap()
    t["wupn"] = nc.dram_tensor("wupn", [128, NH, KVR], BF,
                               kind="ExternalInput").ap()
    t["wupv"] = nc.dram_tensor("wupv", [128, NH, NKV, VD], BF,
                               kind="ExternalInput").ap()
    t["wout"] = nc.dram_tensor("wout", [128, NH, D], BF,
                               kind="ExternalInput").ap()
    t["cosf"] = nc.dram_tensor("cosf", [128, L], BF, kind="ExternalInput").ap()
    t["sinf"] = nc.dram_tensor("sinf", [128, L], BF, kind="ExternalInput").ap()
    t["outT"] = nc.dram_tensor("outT", [D, L], F32, kind="ExternalOutput").ap()

    with tile.TileContext(nc) as tc:
        _emit(tc, t)
    nc.compile()
    return nc


def _emit(tc, t):
    nc = tc.nc
    with ExitStack() as c0:
        c0.enter_context(
            nc.allow_low_precision(reason="bf16 rounding is intentional")
        )
        glob = c0.enter_context(tc.tile_pool(name="glob", bufs=1))
        ps_a = c0.enter_context(tc.tile_pool(name="ps_a", bufs=2, space="PSUM"))
        ps_lo = c0.enter_context(tc.tile_pool(name="ps_lo", bufs=4, space="PSUM"))
        ps_dv = c0.enter_context(tc.tile_pool(name="ps_dv", bufs=2, space="PSUM"))

        # ---- constants ------------------------------------------------
        from concourse import library_config

        nc.gpsimd.load_library(library_config.attnmlp)
        ident = glob.tile([128, 128], F32, tag="ident")
        make_identity(nc, ident)
        identb = glob.tile([128, 128], BF, tag="identb")
        nc.vector.tensor_copy(identb, ident)
        ones_f32 = glob.tile([128, 128], F32, tag="ones32")
        nc.vector.memset(ones_f32, 1.0)
        ones_col = glob.tile([128, 1], BF, tag="ones")
        nc.vector.tensor_copy(ones_col, ones_f32[:, :1])
        eps_t = glob.tile([1, 1], F32, tag="eps")
        nc.vector.memset(eps_t, EPS)
        cosf = glob.tile([128, L], BF, tag="cosf")
        nc.sync.dma_start(out=cosf, in_=t["cosf"])
        sinf = glob.tile([128, L], BF, tag="sinf")
        nc.sync.dma_start(out=sinf, in_=t["sinf"])

        with ExitStack() as c1:
            p1 = c1.enter_context(tc.tile_pool(name="p1", bufs=1))
            kv_lat = p1.tile([128, NKV, L], BF, tag="kvlat")
            k_roped = p1.tile([128, L], BF, tag="kroped")
            # wout prefetch begins early; consumed only in the final phase
            wout_sb = p1.tile([128, NH, D], BF, tag="wout")
            vn_sb = p1.tile([128, NH, L], BF, tag="vnsb")

            with ExitStack() as c2:
                p2 = c2.enter_context(tc.tile_pool(name="p2", bufs=1))
                qT_nope = p2.tile([128, NH, L], BF, tag="qnope")
                q_roped = p2.tile([128, NH // 2, L], BF, tag="qroped")

                _emit_front(tc, t, glob, ps_a, ps_dv,
                            ident, ones_col, eps_t, cosf, sinf,
                            kv_lat, k_roped, qT_nope, q_roped)

                nc.sync.dma_start(out=wout_sb, in_=t["wout"])

                _emit_attn(tc, t, glob, ps_a, ps_lo, ps_dv,
                           identb, ones_col,
                           kv_lat, k_roped, qT_nope, q_roped, vn_sb)

        _emit_outproj(tc, t, glob, ps_a, wout_sb, vn_sb)


def _rms_stats(tc, ps_dv, p3s, lat, nblk, inv_dim, ones_col, eps_t, tag):
    """Return bf16 [128, L] broadcast of 1/rms for latent-major `lat`."""
    nc = tc.nc
    r_row = p3s.tile([1, L], F32, tag=f"rrow{tag}", bufs=1)
    for tk in range(NTOK):
        ts = slice(tk * TOK, (tk + 1) * TOK)
        ps_ssq = ps_dv.tile([1, TOK], F32, tag="dv")
        for lb in range(nblk):
            sq = p3s.tile([128, TOK], BF, tag="sq")
            sl = lat[:, lb, ts]
            nc.vector.tensor_mul(sq, sl, sl)
            nc.tensor.matmul(
                ps_ssq, ones_col, sq,
                start=(lb == 0), stop=(lb == nblk - 1),
            )
        rt = p3s.tile([1, TOK], F32, tag="rt")
        nc.scalar.activation(
            rt, ps_ssq, mybir.ActivationFunctionType.Sqrt,
            bias=eps_t, scale=inv_dim,
        )
        nc.vector.reciprocal_approx_fast(out=r_row[:, ts], in_=rt)
    rbf = p3s.tile([128, L], F32, tag=f"rbf{tag}", bufs=1)
    nc.gpsimd.partition_broadcast(rbf, r_row)
    r_b = p3s.tile([128, L], BF, tag=f"rb{tag}", bufs=1)
    nc.vector.tensor_copy(r_b, rbf)
    return r_b


def _emit_front(tc, t, glob, ps_a, ps_dv, ident, ones_col,
                eps_t, cosf, sinf, kv_lat, k_roped, qT_nope, q_roped):
    """Down projections, RMS norms, k-rope, q up-projection + q-rope."""
    nc = tc.nc
    with ExitStack() as c3:
        p3 = c3.enter_context(tc.tile_pool(name="p3", bufs=1))
        p3s = c3.enter_context(tc.tile_pool(name="p3s", bufs=3))
        q_lat = p3.tile([128, NLAT, L], BF, tag="qlat")
        kr_pair = p3.tile([128, 2, L], BF, tag="krpair")

        # ---- phase 1: down projections, x SBUF-resident --------------
        with ExitStack() as c4:
            p4 = c4.enter_context(tc.tile_pool(name="p4", bufs=1))
            p4s = c4.enter_context(tc.tile_pool(name="p4s", bufs=3))
            x_r = t["x_t"].rearrange("(b p) t -> p b t", p=128)
            x_sb = p4.tile([128, ND, L], BF, tag="xsb")
            wqd0 = p4s.tile([128, ND, 128], BF, tag="wqd")
            nc.sync.dma_start(out=wqd0, in_=t["wqd"][:, 0])
            for ch in range(8):
                cs = slice(ch * 2, ch * 2 + 2)
                nc.sync.dma_start(out=x_sb[:, cs, :], in_=x_r[:, cs, :])

            for lb in range(NLAT):
                if lb == 0:
                    wqd = wqd0
                else:
                    wqd = p4s.tile([128, ND, 128], BF, tag="wqd")
                    nc.sync.dma_start(out=wqd, in_=t["wqd"][:, lb])
                for tk in range(NTOK):
                    ts = slice(tk * TOK, (tk + 1) * TOK)
                    ps = ps_a.tile([128, TOK], F32, tag="a")
                    for db in range(ND):
                        nc.tensor.matmul(
                            ps, wqd[:, db, :], x_sb[:, db, ts],
                            start=(db == 0), stop=(db == ND - 1),
                        )
                    nc.vector.tensor_copy(q_lat[:, lb, ts], ps)

            for mb in range(NKV):
                wkv = p4s.tile([128, ND, 128], BF, tag="wkv")
                nc.sync.dma_start(out=wkv, in_=t["wkvd128"][:, mb])
                for tk in range(NTOK):
                    ts = slice(tk * TOK, (tk + 1) * TOK)
                    ps = ps_a.tile([128, TOK], F32, tag="a")
                    for db in range(ND):
                        nc.tensor.matmul(
                            ps, wkv[:, db, :], x_sb[:, db, ts],
                            start=(db == 0), stop=(db == ND - 1),
                        )
                    nc.vector.tensor_copy(kv_lat[:, mb, ts], ps)

            wkr = p4s.tile([128, ND, ROPE], BF, tag="wkr")
            nc.sync.dma_start(out=wkr, in_=t["wkvdr"])
            for tk in range(NTOK):
                ts = slice(tk * TOK, (tk + 1) * TOK)
                ps = ps_a.tile([128, TOK], F32, tag="a")
                for db in range(ND):
                    nc.tensor.matmul(
                        ps[:ROPE], wkr[:, db, :], x_sb[:, db, ts],
                        start=(db == 0), stop=(db == ND - 1),
                    )
                nc.vector.tensor_copy(kr_pair[:ROPE, 0, ts], ps[:ROPE])

        # ---- phase 1.5: RMS stats; q scale is folded into the up-proj
        # PSUM->SBUF copies, kv is normalized in place -----------------
        rq_b = _rms_stats(tc, ps_dv, p3s, q_lat, NLAT, 1.0 / DQ,
                          ones_col, eps_t, "q")
        rkv_b = _rms_stats(tc, ps_dv, p3s, kv_lat, NKV, 1.0 / KVR,
                           ones_col, eps_t, "kv")
        for lb in range(NKV):
            nc.vector.tensor_mul(kv_lat[:, lb, :], kv_lat[:, lb, :], rkv_b)

        # k rope: swap + rope (rows 0..63), duplicated into both halves
        _rope_half(nc, kr_pair, cosf, sinf, k_roped)
        nc.sync.dma_start(out=k_roped[ROPE:], in_=k_roped[:ROPE])

        # ---- phase 2: q up-projection + q rope (rope packed 2 heads) -
        with ExitStack() as c5:
            p5s = c5.enter_context(tc.tile_pool(name="p5s", bufs=2))
            for hp in range(NH // 2):
                for hh in range(2):
                    h = 2 * hp + hh
                    wn = p5s.tile([128, NLAT, 128], BF, tag="wqun")
                    nc.sync.dma_start(out=wn, in_=t["wqun"][:, h])
                    for tk in range(NTOK):
                        ts = slice(tk * TOK, (tk + 1) * TOK)
                        ps_n = ps_a.tile([128, TOK], F32, tag="a")
                        for lb in range(NLAT):
                            nc.tensor.matmul(
                                ps_n, wn[:, lb, :], q_lat[:, lb, ts],
                                start=(lb == 0), stop=(lb == NLAT - 1),
                            )
                        nc.vector.tensor_mul(
                            qT_nope[:, h, ts], ps_n, rq_b[:, ts]
                        )
                wr = p5s.tile([128, NLAT, 128], BF, tag="wqur")
                nc.sync.dma_start(out=wr, in_=t["wqur"][:, hp])
                q_pair = p5s.tile([128, 2, L], BF, tag="pair")
                for tk in range(NTOK):
                    ts = slice(tk * TOK, (tk + 1) * TOK)
                    ps_rp = ps_a.tile([128, TOK], F32, tag="a")
                    for lb in range(NLAT):
                        nc.tensor.matmul(
                            ps_rp, wr[:, lb, :], q_lat[:, lb, ts],
                            start=(lb == 0), stop=(lb == NLAT - 1),
                        )
                    nc.vector.tensor_mul(q_pair[:, 0, ts], ps_rp, rq_b[:, ts])
                _rope_full(nc, q_pair, cosf, sinf, q_roped[:, hp, :])


def _emit_attn(tc, t, glob, ps_a, ps_lo, ps_dv, identb, ones_col,
               kv_lat, k_roped, qT_nope, q_roped, vn_sb):
    """kv transpose, per-(head, q-tile) attention; v^T kept in SBUF."""
    nc = tc.nc
    with ExitStack() as c6:
        p6 = c6.enter_context(tc.tile_pool(name="p6", bufs=1))
        p6a = c6.enter_context(tc.tile_pool(name="p6a", bufs=2))
        p6s = c6.enter_context(tc.tile_pool(name="p6s", bufs=2))
        p6w = c6.enter_context(tc.tile_pool(name="p6w", bufs=3))

        # transpose normalized kv_lat -> token-major
        kv_tok = p6.tile([128, NKB, KVR], BF, tag="kvtok")
        for kb in range(NKB):
            ps = ps_a.tile([128, KVR], BF, tag="a")
            for lb in range(NKV):
                nc.tensor.transpose(
                    ps[:, lb * 128 : (lb + 1) * 128],
                    kv_lat[:, lb, kb * 128 : (kb + 1) * 128],
                    identb,
                )
            nc.vector.tensor_copy(kv_tok[:, kb, :], ps)

        for h in range(NH):
            hb = (h % 2) * 64
            wn = p6s.tile([128, KVR], BF, tag="wupn")
            nc.sync.dma_start(out=wn, in_=t["wupn"][:, h])
            wv = p6s.tile([128, NKV, VD], BF, tag="wupv")
            nc.sync.dma_start(out=wv, in_=t["wupv"][:, h])

            q_abs = p6a.tile([128, NKV, L], BF, tag="qabs")
            for mb in range(NKV):
                for tk in range(NTOK):
                    ts = slice(tk * TOK, (tk + 1) * TOK)
                    ps = ps_a.tile([128, TOK], F32, tag="a")
                    nc.tensor.matmul(
                        ps, wn[:, mb * 128 : (mb + 1) * 128],
                        qT_nope[:, h, ts],
                    )
                    nc.vector.tensor_copy(q_abs[:, mb, ts], ps)

            for tk in range(NTOK):
                ts = slice(tk * TOK, (tk + 1) * TOK)
                nkb = (tk + 1) * (TOK // KB)
                ps_d = ps_dv.tile([1, TOK], F32, tag="dv")
                ps_los = [
                    ps_lo.tile([128, TOK], F32, tag="lo", name=f"pslo{i}")
                    for i in range(NKV)
                ]
                for kb in range(nkb):
                    ks = slice(kb * 128, (kb + 1) * 128)
                    off = max(0, kb * KB - tk * TOK)
                    qs = slice(tk * TOK + off, (tk + 1) * TOK)
                    ps_s = ps_a.tile([128, TOK], F32, tag="a")
                    for lb in range(NKV):
                        nc.tensor.matmul(
                            ps_s[:, off:], kv_lat[:, lb, ks],
                            q_abs[:, lb, qs],
                            start=(lb == 0), stop=False,
                        )
                    nc.tensor.matmul(
                        ps_s[:, off:], k_roped[hb : hb + ROPE, ks],
                        q_roped[hb : hb + ROPE, h // 2, qs],
                        start=False, stop=True,
                    )
                    e_t = p6w.tile([128, TOK], BF, tag="e")
                    nc.scalar.activation(
                        e_t[:, off:], ps_s[:, off:],
                        mybir.ActivationFunctionType.Exp, scale=SCALE,
                    )
                    if kb * KB >= tk * TOK:
                        nc.gpsimd.affine_select(
                            out=e_t[:, off : off + KB],
                            in_=e_t[:, off : off + KB],
                            pattern=[[1, KB]],
                            compare_op=mybir.AluOpType.is_ge,
                            fill=0.0,
                            base=0,
                            channel_multiplier=-1,
                        )
                    nc.tensor.matmul(
                        ps_d[:, off:], ones_col, e_t[:, off:],
                        start=(kb == 0), stop=(kb == nkb - 1),
                    )
                    for lb in range(NKV):
                        nc.tensor.matmul(
                            ps_los[lb][:, off:],
                            kv_tok[:, kb, lb * 128 : (lb + 1) * 128],
                            e_t[:, off:],
                            start=(kb == 0), stop=(kb == nkb - 1),
                        )
                rd = p6w.tile([1, TOK], F32, tag="rd")
                nc.vector.reciprocal_approx_fast(out=rd, in_=ps_d)
                rb_f = p6w.tile([128, TOK], F32, tag="rbf")
                nc.gpsimd.partition_broadcast(rb_f, rd)
                lo_t = p6w.tile([128, NKV, TOK], BF, tag="lot", bufs=2)
                for lb in range(NKV):
                    nc.scalar.copy(lo_t[:, lb, :], ps_los[lb])
                ps_v = ps_dv.tile([128, TOK], F32, tag="dv")
                for lb in range(NKV):
                    nc.tensor.matmul(
                        ps_v, wv[:, lb, :], lo_t[:, lb, :],
                        start=(lb == 0), stop=(lb == NKV - 1),
                    )
                nc.vector.tensor_mul(vn_sb[:, h, ts], ps_v, rb_f)


def _emit_outproj(tc, t, glob, ps_a, wout_sb, vn_sb):
    """out^T [D, L] = sum_h Wout_h^T-blocks @ v_norm_h^T."""
    nc = tc.nc
    with ExitStack() as c7:
        p7s = c7.enter_context(tc.tile_pool(name="p7s", bufs=3))
        for db in range(D // 128):
            for tk in range(NTOK):
                ts = slice(tk * TOK, (tk + 1) * TOK)
                ps = ps_a.tile([128, TOK], F32, tag="a")
                for h in range(NH):
                    nc.tensor.matmul(
                        ps, wout_sb[:, h, db * 128 : (db + 1) * 128],
                        vn_sb[:, h, ts],
                        start=(h == 0), stop=(h == NH - 1),
                    )
                o_t = p7s.tile([128, TOK], F32, tag="o")
                nc.vector.tensor_copy(o_t, ps)
                nc.sync.dma_start(
                    out=t["outT"][db * 128 : (db + 1) * 128, ts], in_=o_t
                )


def _rope_half(nc, pair, cosf, sinf, out):
    """pair[:64,0,:] = v in split re/im layout (re rows 0..31, im rows
    32..63); fill pair[:64,1,:] with the partner rows, then rope into
    out rows [0, 64)."""
    nc.sync.dma_start(out=pair[0:32, 1, :], in_=pair[32:64, 0, :])
    nc.sync.dma_start(out=pair[32:64, 1, :], in_=pair[0:32, 0, :])
    a = pair[:ROPE, 0, :]
    b = pair[:ROPE, 1, :]
    ob = out[:ROPE]
    # out = a*cos + b*sinf'  (sign of the swap folded into sinf)
    nc.vector.tensor_mul(ob, a, cosf[:ROPE])
    nc.vector.tensor_mul(pair[:ROPE, 0, :], b, sinf[:ROPE])
    nc.vector.tensor_add(ob, ob, pair[:ROPE, 0, :])


def _rope_full(nc, pair, cosf, sinf, out):
    """Rope both 64-row halves of pair[:,0,:] (two heads packed) into
    out [128, L]."""
    nc.sync.dma_start(out=pair[0:32, 1, :], in_=pair[32:64, 0, :])
    nc.sync.dma_start(out=pair[32:64, 1, :], in_=pair[0:32, 0, :])
    nc.sync.dma_start(out=pair[64:96, 1, :], in_=pair[96:128, 0, :])
    nc.sync.dma_start(out=pair[96:128, 1, :], in_=pair[64:96, 0, :])
    a = pair[:, 0, :]
    b = pair[:, 1, :]
    nc.vector.tensor_mul(out, a, cosf)
    nc.vector.tensor_mul(pair[:, 0, :], b, sinf)
    nc.vector.tensor_add(out, out, pair[:, 0, :])


# ======================================================================
# host side
# ======================================================================

_NC_CACHE = {}


def _get_nc():
    if "nc" not in _NC_CACHE:
        _NC_CACHE["nc"] = build_nc()
    return _NC_CACHE["nc"]


def _prep_shared(inputs):
    wq_down = np.asarray(inputs["Wq_down"], np.float32)
    wq_up = np.asarray(inputs["Wq_up"], np.float32)
    wkv_down = np.asarray(inputs["Wkv_down"], np.float32)
    wkv_up = np.asarray(inputs["Wkv_up"], np.float32)
    wout = np.asarray(inputs["Wout"], np.float32)
    rms_q_w = np.asarray(inputs["rms_q_w"], np.float32)
    rms_kv_w = np.asarray(inputs["rms_kv_w"], np.float32)
    freq = np.asarray(inputs["freq_cis"], np.float32)  # [L, 32, 2]

    def b16(a):
        return np.ascontiguousarray(a.astype(BNP))

    # split re/im layout for all rope dims: re parts first, then im parts
    rope_perm = np.concatenate(
        [np.arange(0, ROPE, 2), np.arange(1, ROPE, 2)]
    )  # [64]

    # wqd [128, NLAT, ND, 128]: arr[p, lb, b, c] = wq_down.T[b*128+p, lb*128+c]
    wqd_t = wq_down.T  # [D, DQ]
    wqd_h = wqd_t.reshape(ND, 128, NLAT, 128).transpose(1, 2, 0, 3)

    wkv_down_p = wkv_down.copy()
    wkv_down_p[KVR:] = wkv_down[KVR:][rope_perm]
    wkvd_t = wkv_down_p.T  # [D, 576]
    a = wkvd_t.reshape(ND, 128, KVR + ROPE)
    wkvd128_h = a[:, :, :KVR].reshape(ND, 128, NKV, 128).transpose(1, 2, 0, 3)
    wkvdr_h = a[:, :, KVR:].transpose(1, 0, 2)  # [128, ND, 64]

    # rope tables (dim-major, split re/im, duplicated partition halves)
    cos = freq[:, :, 0].T  # [32, L]
    sin = freq[:, :, 1].T
    cosf64 = np.vstack([cos, cos])  # [64, L]
    sinf64 = np.vstack([-sin, sin])
    cosf = np.vstack([cosf64, cosf64])  # [128, L]
    sinf = np.vstack([sinf64, sinf64])

    wq_up3 = (wq_up * rms_q_w[None, :]).reshape(H, HD, DQ)
    wq_up3 = np.concatenate(
        [wq_up3[:, :NOPE, :], wq_up3[:, NOPE:, :][:, rope_perm, :]], axis=1
    )
    wkv_up3 = wkv_up.reshape(H, NOPE + VD, KVR)
    wout3 = wout.reshape(D, H, VD)

    per_g = []
    for g in range(2):
        hs = list(range(g * NH, (g + 1) * NH))
        wn3 = wq_up3[hs, :NOPE, :]  # [8, 128, DQ]
        # wqun [128, NH, NLAT, 128]: arr[p, h, lb, c] = wn3[h, c, lb*128+p]
        wqun_h = (
            wn3.transpose(2, 0, 1)
            .reshape(NLAT, 128, NH, NOPE)
            .transpose(1, 2, 0, 3)
        )
        rg = wq_up3[hs, NOPE:, :]  # [8, 64, DQ]
        bpair = np.stack(
            [np.concatenate([rg[2 * i], rg[2 * i + 1]], axis=0)
             for i in range(NH // 2)]
        )  # [4, 128, DQ]
        wqur_h = (
            bpair.transpose(2, 0, 1)
            .reshape(NLAT, 128, NH // 2, 128)
            .transpose(1, 2, 0, 3)
        )
        wupn = wkv_up3[hs, :NOPE, :] * rms_kv_w[None, None, :]  # [8,128,512]
        wupn_h = wupn.transpose(1, 0, 2)  # [128, NH, 512]
        wupv = wkv_up3[hs, NOPE:, :] * rms_kv_w[None, None, :]  # [8,128,512]
        # wupv [128, NH, NKV, VD]: arr[p, h, lb, v] = wupv[h, v, lb*128+p]
        wupv_h = (
            wupv.transpose(2, 0, 1)
            .reshape(NKV, 128, NH, VD)
            .transpose(1, 2, 0, 3)
        )
        wout_t = wout3[:, hs, :].transpose(1, 2, 0).reshape(NH * VD, D)
        wout_h = wout_t.reshape(NH, 128, D).transpose(1, 0, 2)  # [128,NH,D]
        per_g.append(
            {
                "wqd": b16(wqd_h),
                "wkvd128": b16(wkvd128_h),
                "wkvdr": b16(wkvdr_h),
                "wqun": b16(wqun_h),
                "wqur": b16(wqur_h),
                "wupn": b16(wupn_h),
                "wupv": b16(wupv_h),
                "wout": b16(wout_h),
                "cosf": b16(cosf),
                "sinf": b16(sinf),
            }
        )
    return per_g


def make_in_maps(inputs):
    x = np.asarray(inputs["x"], np.float32)
    per_g = _prep_shared(inputs)
    in_maps = []
    for c in range(N_CORES):
        b, g = c // 2, c % 2
        m = dict(per_g[g])
        m["x_t"] = np.ascontiguousarray(x[b].T.astype(BNP))
        in_maps.append(m)
    return in_maps


def kernel(**inputs):
    nc = _get_nc()
    in_maps = make_in_maps(inputs)
    res = bass_utils.run_bass_kernel_spmd(
        nc, in_maps, core_ids=list(range(N_CORES))
    ).results
    out = np.empty((B, L, D), np.float32)
    for b in range(B):
        out[b] = (res[2 * b]["outT"] + res[2 * b + 1]["outT"]).T
    return out
